# revision 17
# baseline (speedup 1.0000x reference)
"""Trainium2 Bass kernel: 3-level 2D DWT (depthwise 2x2 stride-2 conv with
separable 2-tap filters, edge padding degenerates to none for even sizes).

Data-parallel over the 1024 (batch*channel) images: 128 images per NeuronCore
across 8 cores. On each core the SBUF partition dim indexes the 128 images and
the free dim holds pixels. Input is streamed in bands of BR=32 rows; since the
2-tap stride-2 transform is non-overlapping, each band flows through all 3
levels entirely on-chip (level-1 details DMA out per band, level-2 details per
band, level-3 results accumulate in SBUF and DMA at the end).

Per level, with E/O = even/odd rows and (for a 2-tap filter [f0, f1]):
    Vlo = l0*E + l1*O          Vhi = h0*E + h1*O      (vertical pass)
    ll  = l0*Vlo_e + l1*Vlo_o  hl = h0*Vlo_e + h1*Vlo_o
    lh  = l0*Vhi_e + l1*Vhi_o  hh = h0*Vhi_e + h1*Vhi_o  (horizontal pass)
Each a*X + b*Y runs as ONE fused scalar_tensor_tensor on VectorE via the
ratio form a*(X + (b/a)*Y) with scales folded forward along the ll chain;
detail outputs get their exact scale via a ScalarE prescale. The kernel is
DMA-bound (memory regime): HBM traffic is the irreducible 33.5MB in +
33.5MB out per core, measured ~335 GB/s sustained.
"""

import numpy as np

_B, _C, _H, _W = 16, 64, 256, 256
_NCORES = 8
_P = 128  # partitions = images per core
_BR = 32  # input rows per band


def build_dwt_nc(l0, l1, h0, h1, H, W, BR):
    """Build the per-core Bass module. Input x: [128, H*W] fp32 (one image per
    partition, row-major). Outputs: lh/hl/hh at levels 1..3 plus l3."""
    import concourse.bacc as bacc
    import concourse.mybir as mybir
    from concourse import tile

    dt = mybir.dt.float32
    MULT = mybir.AluOpType.mult
    ADD = mybir.AluOpType.add
    P = _P

    assert BR % 8 == 0 and H % BR == 0 and W % 8 == 0

    nc = bacc.Bacc("TRN2", target_bir_lowering=False, debug=False)
    x = nc.dram_tensor("x", [P, H * W], dt, kind="ExternalInput")

    dims = {lvl: (H >> lvl, W >> lvl) for lvl in (1, 2, 3)}
    outs = {}
    for lvl in (1, 2, 3):
        h_, w_ = dims[lvl]
        for nm in ("lh", "hl", "hh"):
            outs[f"{nm}{lvl}"] = nc.dram_tensor(
                f"{nm}{lvl}", [P, h_ * w_], dt, kind="ExternalOutput"
            )
    outs["l3"] = nc.dram_tensor("l3", [P, dims[3][0] * dims[3][1]], dt,
                                kind="ExternalOutput")

    nbands = H // BR

    with tile.TileContext(nc) as tc:
        with (
            tc.tile_pool(name="xin", bufs=2) as xp,
            tc.tile_pool(name="uwork", bufs=3) as up,
            tc.tile_pool(name="llp", bufs=1) as llp,
            tc.tile_pool(name="det1", bufs=2) as detp,
            tc.tile_pool(name="det2", bufs=1) as det2p,
            tc.tile_pool(name="acc", bufs=1) as accp,
        ):
            r3f, c3f = dims[3]
            l3_acc = {nm: accp.tile([P, r3f * c3f], dt, tag=f"acc_{nm}",
                                    name=f"acc_{nm}")
                      for nm in ("l3", "lh3", "hl3", "hh3")}

            # Ratio form: a*E + b*O = a*(E + (b/a)*O). Vertical passes and the
            # ll chain keep results UNSCALED (scale folded forward); details
            # get the accumulated scale baked into their scalar-engine
            # prescale. Requires l0, h0 != 0 (true for the Haar bank and any
            # real filter; asserted in kernel()).
            r_l = l1 / l0
            r_h = h1 / h0
            # sigma[lvl] = scale of the unscaled ll' input entering level lvl
            sigma = {1: 1.0, 2: l0 * l0, 3: (l0 * l0) ** 2}

            def do_level(src, R, C, lvl, dst_ll, dst_lh, dst_hl, dst_hh):
                # src: AP [P, R*C] (unscaled by sigma[lvl]); dsts: APs
                # [P, (R//2)*(C//2)]. dst_ll left unscaled; details exact.
                r, c = R // 2, C // 2
                s = sigma[lvl]
                pairs = src.rearrange("p (r two c) -> p r two c", two=2, c=C)
                E = pairs[:, :, 0, :]
                O = pairs[:, :, 1, :]

                # vertical: Vlo' = E + r_l*O ; Vhi' = E + r_h*O
                u = up.tile([P, r * C], dt, tag=f"u{C}", name=f"u{C}a")
                uv = u[:].rearrange("p (r c) -> p r c", c=C)
                nc.vector.scalar_tensor_tensor(uv, O, r_l, E, MULT, ADD)
                u2 = up.tile([P, r * C], dt, tag=f"u{C}", name=f"u{C}b")
                u2v = u2[:].rearrange("p (r c) -> p r c", c=C)
                nc.vector.scalar_tensor_tensor(u2v, O, r_h, E, MULT, ADD)

                def cols(ap):
                    veo = ap.rearrange("p (r c two) -> p r c two", two=2, c=c)
                    return veo[:, :, :, 0], veo[:, :, :, 1]

                Ve, Vo = cols(u[:])
                We, Wo = cols(u2[:])

                # ll' = Ve + r_l*Vo (unscaled by s*l0^2, folded forward)
                ll_v = dst_ll.rearrange("p (r c) -> p r c", c=c)
                nc.vector.scalar_tensor_tensor(ll_v, Vo, r_l, Ve, MULT, ADD)
                if lvl == 3:
                    # final ll-chain scale applied per band (l3 is an output)
                    nc.scalar.mul(ll_v, ll_v, s * l0 * l0)

                # details: d = (s*f0*v0)*Ve + (s*f0*v1)*Vo, exact scales
                for dst, Xe, Xo, f0, f1 in (
                    (dst_hl, Ve, Vo, l0 * h0, l0 * h1),
                    (dst_lh, We, Wo, h0 * l0, h0 * l1),
                    (dst_hh, We, Wo, h0 * h0, h0 * h1),
                ):
                    dv = dst.rearrange("p (r c) -> p r c", c=c)
                    nc.scalar.mul(dv, Xe, s * f0)
                    nc.vector.scalar_tensor_tensor(dv, Xo, s * f1, dv, MULT, ADD)

            # Taper the first and last bands so the pipeline ramps in
            # quickly (vector starts after a 2MB DMA instead of 4MB) and the
            # post-compute DMA tail is halved.
            if nbands >= 2 and BR // 2 >= 8:
                band_rows = [BR // 2, BR // 2] + [BR] * (nbands - 2) + [BR // 2, BR // 2]
            else:
                band_rows = [BR] * nbands
            assert sum(band_rows) == H
            row0 = 0
            l3_flush_done = 0
            l3_rows_total = 0
            for bi, R in enumerate(band_rows):
                X = xp.tile([P, R * W], dt, tag="X", name="X")
                nc.sync.dma_start(X[:], x[:, row0 * W:(row0 + R) * W])

                r1b, c1 = R // 2, W // 2
                r2b, c2 = r1b // 2, c1 // 2
                r3b, c3 = r2b // 2, c2 // 2
                ll1 = llp.tile([P, r1b * c1], dt, tag="ll1", name="ll1")
                d1 = {nm: detp.tile([P, r1b * c1], dt, tag=f"{nm}1", name=f"{nm}1t")
                      for nm in ("lh", "hl", "hh")}
                do_level(X[:], R, W, 1, ll1[:], d1["lh"][:], d1["hl"][:], d1["hh"][:])
                o1 = (row0 // 2) * c1
                for nm in ("lh", "hl", "hh"):
                    nc.sync.dma_start(outs[f"{nm}1"][:, o1:o1 + r1b * c1],
                                      d1[nm][:])

                ll2 = llp.tile([P, r2b * c2], dt, tag="ll2", name="ll2")
                d2 = {nm: det2p.tile([P, r2b * c2], dt, tag=f"{nm}2", name=f"{nm}2t")
                      for nm in ("lh", "hl", "hh")}
                do_level(ll1[:], r1b, c1, 2, ll2[:], d2["lh"][:], d2["hl"][:], d2["hh"][:])
                o2 = (row0 // 4) * c2
                for nm in ("lh", "hl", "hh"):
                    nc.sync.dma_start(outs[f"{nm}2"][:, o2:o2 + r2b * c2],
                                      d2[nm][:])

                o3 = (row0 // 8) * c3
                sl = slice(o3, o3 + r3b * c3)
                do_level(ll2[:], r2b, c2, 3, l3_acc["l3"][:, sl], l3_acc["lh3"][:, sl],
                         l3_acc["hl3"][:, sl], l3_acc["hh3"][:, sl])
                l3_rows_total = o3 + r3b * c3

                # flush level-3 accumulators halfway and at the end so the
                # final DMAs are small and overlap earlier compute
                if bi == len(band_rows) // 2 - 1 or bi == len(band_rows) - 1:
                    for nm in ("l3", "lh3", "hl3", "hh3"):
                        nc.sync.dma_start(outs[nm][:, l3_flush_done:l3_rows_total],
                                          l3_acc[nm][:, l3_flush_done:l3_rows_total])
                    l3_flush_done = l3_rows_total
                row0 += R

    nc.compile()
    return nc


_CACHE = {}


def _get_nc(key, l0, l1, h0, h1, H, W):
    if key not in _CACHE:
        _CACHE[key] = build_dwt_nc(l0, l1, h0, h1, H, W, _BR)
    return _CACHE[key]


def kernel(x, dec_lo, dec_hi):
    from concourse.bass_utils import run_bass_kernel_spmd

    x = np.ascontiguousarray(np.asarray(x), dtype=np.float32)
    lo = np.asarray(dec_lo, dtype=np.float32).reshape(-1)
    hi = np.asarray(dec_hi, dtype=np.float32).reshape(-1)
    l0, l1 = float(lo[0]), float(lo[1])
    h0, h1 = float(hi[0]), float(hi[1])
    assert abs(l0) > 1e-20 and abs(h0) > 1e-20, (l0, h0)
    B, C, H, W = x.shape
    assert (B * C) == _P * _NCORES, (B, C)

    nc = _get_nc((l0, l1, h0, h1, H, W), l0, l1, h0, h1, H, W)

    xf = x.reshape(B * C, H * W)
    in_maps = [{"x": xf[c * _P:(c + 1) * _P]} for c in range(_NCORES)]
    res = run_bass_kernel_spmd(nc, in_maps, core_ids=list(range(_NCORES))).results

    def g(name, lvl):
        h_, w_ = H >> lvl, W >> lvl
        return np.concatenate([res[c][name] for c in range(_NCORES)],
                              axis=0).reshape(B, C, h_, w_)

    l3 = g("l3", 3)
    lh3, hl3, hh3 = g("lh3", 3), g("hl3", 3), g("hh3", 3)
    lh2, hl2, hh2 = g("lh2", 2), g("hl2", 2), g("hh2", 2)
    lh1, hl1, hh1 = g("lh1", 1), g("hl1", 1), g("hh1", 1)
    return (l3, (lh3, hl3, hh3), (lh2, hl2, hh2), (lh1, hl1, hh1))


# revision 18
# speedup vs baseline: 1.0095x; 1.0095x over previous
"""Trainium2 Bass kernel: 3-level 2D DWT (depthwise 2x2 stride-2 conv with
separable 2-tap filters, edge padding degenerates to none for even sizes).

Data-parallel over the 1024 (batch*channel) images: 128 images per NeuronCore
across 8 cores. On each core the SBUF partition dim indexes the 128 images and
the free dim holds pixels. Input is streamed in bands of BR=32 rows; since the
2-tap stride-2 transform is non-overlapping, each band flows through all 3
levels entirely on-chip (level-1 details DMA out per band, level-2 details per
band, level-3 results accumulate in SBUF and DMA at the end).

Per level, with E/O = even/odd rows and (for a 2-tap filter [f0, f1]):
    Vlo = l0*E + l1*O          Vhi = h0*E + h1*O      (vertical pass)
    ll  = l0*Vlo_e + l1*Vlo_o  hl = h0*Vlo_e + h1*Vlo_o
    lh  = l0*Vhi_e + l1*Vhi_o  hh = h0*Vhi_e + h1*Vhi_o  (horizontal pass)
Each a*X + b*Y runs as ONE fused scalar_tensor_tensor on VectorE via the
ratio form a*(X + (b/a)*Y) with scales folded forward along the ll chain;
detail outputs get their exact scale via a ScalarE prescale. The kernel is
DMA-bound (memory regime): HBM traffic is the irreducible 33.5MB in +
33.5MB out per core, measured ~335 GB/s sustained.
"""

import numpy as np

_B, _C, _H, _W = 16, 64, 256, 256
_NCORES = 8
_P = 128  # partitions = images per core
_BR = 32  # input rows per band


def build_dwt_nc(l0, l1, h0, h1, H, W, BR):
    """Build the per-core Bass module. Input x: [128, H*W] fp32 (one image per
    partition, row-major). Outputs: lh/hl/hh at levels 1..3 plus l3."""
    import concourse.bacc as bacc
    import concourse.mybir as mybir
    from concourse import tile

    dt = mybir.dt.float32
    MULT = mybir.AluOpType.mult
    ADD = mybir.AluOpType.add
    P = _P

    assert BR % 8 == 0 and H % BR == 0 and W % 8 == 0

    nc = bacc.Bacc("TRN2", target_bir_lowering=False, debug=False)
    x = nc.dram_tensor("x", [P, H * W], dt, kind="ExternalInput")

    dims = {lvl: (H >> lvl, W >> lvl) for lvl in (1, 2, 3)}
    outs = {}
    for lvl in (1, 2, 3):
        h_, w_ = dims[lvl]
        for nm in ("lh", "hl", "hh"):
            outs[f"{nm}{lvl}"] = nc.dram_tensor(
                f"{nm}{lvl}", [P, h_ * w_], dt, kind="ExternalOutput"
            )
    outs["l3"] = nc.dram_tensor("l3", [P, dims[3][0] * dims[3][1]], dt,
                                kind="ExternalOutput")

    nbands = H // BR

    with tile.TileContext(nc) as tc:
        with (
            tc.tile_pool(name="xin", bufs=2) as xp,
            tc.tile_pool(name="uwork", bufs=2) as up,
            tc.tile_pool(name="llp", bufs=1) as llp,
            tc.tile_pool(name="det1", bufs=2) as detp,
            tc.tile_pool(name="det2", bufs=1) as det2p,
            tc.tile_pool(name="acc", bufs=1) as accp,
        ):
            r3f, c3f = dims[3]
            l3_acc = {nm: accp.tile([P, r3f * c3f], dt, tag=f"acc_{nm}",
                                    name=f"acc_{nm}")
                      for nm in ("l3", "lh3", "hl3", "hh3")}

            # Ratio form: a*E + b*O = a*(E + (b/a)*O). Vertical passes and the
            # ll chain keep results UNSCALED (scale folded forward); details
            # get the accumulated scale baked into their scalar-engine
            # prescale. Requires l0, h0 != 0 (true for the Haar bank and any
            # real filter; asserted in kernel()).
            r_l = l1 / l0
            r_h = h1 / h0
            # sigma[lvl] = scale of the unscaled ll' input entering level lvl
            sigma = {1: 1.0, 2: l0 * l0, 3: (l0 * l0) ** 2}

            def do_level(src, R, C, lvl, dst_ll, dst_lh, dst_hl, dst_hh):
                # src: AP [P, R*C] (unscaled by sigma[lvl]); dsts: APs
                # [P, (R//2)*(C//2)]. dst_ll left unscaled; details exact.
                r, c = R // 2, C // 2
                s = sigma[lvl]
                pairs = src.rearrange("p (r two c) -> p r two c", two=2, c=C)
                E = pairs[:, :, 0, :]
                O = pairs[:, :, 1, :]

                # vertical: Vlo' = E + r_l*O ; Vhi' = E + r_h*O
                u = up.tile([P, r * C], dt, tag=f"u{C}", name=f"u{C}a")
                uv = u[:].rearrange("p (r c) -> p r c", c=C)
                nc.vector.scalar_tensor_tensor(uv, O, r_l, E, MULT, ADD)
                u2 = up.tile([P, r * C], dt, tag=f"u{C}", name=f"u{C}b")
                u2v = u2[:].rearrange("p (r c) -> p r c", c=C)
                nc.vector.scalar_tensor_tensor(u2v, O, r_h, E, MULT, ADD)

                def cols(ap):
                    veo = ap.rearrange("p (r c two) -> p r c two", two=2, c=c)
                    return veo[:, :, :, 0], veo[:, :, :, 1]

                Ve, Vo = cols(u[:])
                We, Wo = cols(u2[:])

                # ll' = Ve + r_l*Vo (unscaled by s*l0^2, folded forward)
                ll_v = dst_ll.rearrange("p (r c) -> p r c", c=c)
                nc.vector.scalar_tensor_tensor(ll_v, Vo, r_l, Ve, MULT, ADD)
                if lvl == 3:
                    # final ll-chain scale applied per band (l3 is an output)
                    nc.scalar.mul(ll_v, ll_v, s * l0 * l0)

                # details: d = (s*f0*v0)*Ve + (s*f0*v1)*Vo, exact scales
                for dst, Xe, Xo, f0, f1 in (
                    (dst_hl, Ve, Vo, l0 * h0, l0 * h1),
                    (dst_lh, We, Wo, h0 * l0, h0 * l1),
                    (dst_hh, We, Wo, h0 * h0, h0 * h1),
                ):
                    dv = dst.rearrange("p (r c) -> p r c", c=c)
                    nc.scalar.mul(dv, Xe, s * f0)
                    nc.vector.scalar_tensor_tensor(dv, Xo, s * f1, dv, MULT, ADD)

            # Taper the first and last bands so the pipeline ramps in
            # quickly (vector starts after a 2MB DMA instead of 4MB) and the
            # post-compute DMA tail is halved.
            if nbands >= 2 and BR // 2 >= 8:
                band_rows = [BR // 2, BR // 2] + [BR] * (nbands - 2) + [BR // 2, BR // 2]
            else:
                band_rows = [BR] * nbands
            assert sum(band_rows) == H
            row0 = 0
            l3_flush_done = 0
            l3_rows_total = 0
            for bi, R in enumerate(band_rows):
                X = xp.tile([P, R * W], dt, tag="X", name="X")
                nc.sync.dma_start(X[:], x[:, row0 * W:(row0 + R) * W])

                r1b, c1 = R // 2, W // 2
                r2b, c2 = r1b // 2, c1 // 2
                r3b, c3 = r2b // 2, c2 // 2
                ll1 = llp.tile([P, r1b * c1], dt, tag="ll1", name="ll1")
                d1 = {nm: detp.tile([P, r1b * c1], dt, tag=f"{nm}1", name=f"{nm}1t")
                      for nm in ("lh", "hl", "hh")}
                do_level(X[:], R, W, 1, ll1[:], d1["lh"][:], d1["hl"][:], d1["hh"][:])
                o1 = (row0 // 2) * c1
                for nm in ("lh", "hl", "hh"):
                    nc.sync.dma_start(outs[f"{nm}1"][:, o1:o1 + r1b * c1],
                                      d1[nm][:])

                ll2 = llp.tile([P, r2b * c2], dt, tag="ll2", name="ll2")
                d2 = {nm: det2p.tile([P, r2b * c2], dt, tag=f"{nm}2", name=f"{nm}2t")
                      for nm in ("lh", "hl", "hh")}
                do_level(ll1[:], r1b, c1, 2, ll2[:], d2["lh"][:], d2["hl"][:], d2["hh"][:])
                o2 = (row0 // 4) * c2
                for nm in ("lh", "hl", "hh"):
                    nc.sync.dma_start(outs[f"{nm}2"][:, o2:o2 + r2b * c2],
                                      d2[nm][:])

                o3 = (row0 // 8) * c3
                sl = slice(o3, o3 + r3b * c3)
                do_level(ll2[:], r2b, c2, 3, l3_acc["l3"][:, sl], l3_acc["lh3"][:, sl],
                         l3_acc["hl3"][:, sl], l3_acc["hh3"][:, sl])
                l3_rows_total = o3 + r3b * c3

                # flush level-3 accumulators halfway and at the end so the
                # final DMAs are small and overlap earlier compute
                if bi == len(band_rows) // 2 - 1 or bi == len(band_rows) - 1:
                    for nm in ("l3", "lh3", "hl3", "hh3"):
                        nc.sync.dma_start(outs[nm][:, l3_flush_done:l3_rows_total],
                                          l3_acc[nm][:, l3_flush_done:l3_rows_total])
                    l3_flush_done = l3_rows_total
                row0 += R

    nc.compile()
    return nc


_CACHE = {}


def _get_nc(key, l0, l1, h0, h1, H, W):
    if key not in _CACHE:
        _CACHE[key] = build_dwt_nc(l0, l1, h0, h1, H, W, _BR)
    return _CACHE[key]


def kernel(x, dec_lo, dec_hi):
    from concourse.bass_utils import run_bass_kernel_spmd

    x = np.ascontiguousarray(np.asarray(x), dtype=np.float32)
    lo = np.asarray(dec_lo, dtype=np.float32).reshape(-1)
    hi = np.asarray(dec_hi, dtype=np.float32).reshape(-1)
    l0, l1 = float(lo[0]), float(lo[1])
    h0, h1 = float(hi[0]), float(hi[1])
    assert abs(l0) > 1e-20 and abs(h0) > 1e-20, (l0, h0)
    B, C, H, W = x.shape
    assert (B * C) == _P * _NCORES, (B, C)

    nc = _get_nc((l0, l1, h0, h1, H, W), l0, l1, h0, h1, H, W)

    xf = x.reshape(B * C, H * W)
    in_maps = [{"x": xf[c * _P:(c + 1) * _P]} for c in range(_NCORES)]
    res = run_bass_kernel_spmd(nc, in_maps, core_ids=list(range(_NCORES))).results

    def g(name, lvl):
        h_, w_ = H >> lvl, W >> lvl
        return np.concatenate([res[c][name] for c in range(_NCORES)],
                              axis=0).reshape(B, C, h_, w_)

    l3 = g("l3", 3)
    lh3, hl3, hh3 = g("lh3", 3), g("hl3", 3), g("hh3", 3)
    lh2, hl2, hh2 = g("lh2", 2), g("hl2", 2), g("hh2", 2)
    lh1, hl1, hh1 = g("lh1", 1), g("hl1", 1), g("hh1", 1)
    return (l3, (lh3, hl3, hh3), (lh2, hl2, hh2), (lh1, hl1, hh1))


# revision 19
# speedup vs baseline: 1.0136x; 1.0041x over previous
"""Trainium2 Bass kernel: 3-level 2D DWT (depthwise 2x2 stride-2 conv with
separable 2-tap filters, edge padding degenerates to none for even sizes).

Data-parallel over the 1024 (batch*channel) images: 128 images per NeuronCore
across 8 cores. On each core the SBUF partition dim indexes the 128 images and
the free dim holds pixels. Input is streamed in bands of BR=32 rows; since the
2-tap stride-2 transform is non-overlapping, each band flows through all 3
levels entirely on-chip (level-1 details DMA out per band, level-2 details per
band, level-3 results accumulate in SBUF and DMA at the end).

Per level, with E/O = even/odd rows and (for a 2-tap filter [f0, f1]):
    Vlo = l0*E + l1*O          Vhi = h0*E + h1*O      (vertical pass)
    ll  = l0*Vlo_e + l1*Vlo_o  hl = h0*Vlo_e + h1*Vlo_o
    lh  = l0*Vhi_e + l1*Vhi_o  hh = h0*Vhi_e + h1*Vhi_o  (horizontal pass)
Each a*X + b*Y runs as ONE fused scalar_tensor_tensor on VectorE via the
ratio form a*(X + (b/a)*Y) with scales folded forward along the ll chain;
detail outputs get their exact scale via a ScalarE prescale. The kernel is
DMA-bound (memory regime): HBM traffic is the irreducible 33.5MB in +
33.5MB out per core, measured ~335 GB/s sustained.
"""

import numpy as np

_B, _C, _H, _W = 16, 64, 256, 256
_NCORES = 8
_P = 128  # partitions = images per core
_BR = 32  # input rows per band


def build_dwt_nc(l0, l1, h0, h1, H, W, BR):
    """Build the per-core Bass module. Input x: [128, H*W] fp32 (one image per
    partition, row-major). Outputs: lh/hl/hh at levels 1..3 plus l3."""
    import concourse.bacc as bacc
    import concourse.mybir as mybir
    from concourse import tile

    dt = mybir.dt.float32
    MULT = mybir.AluOpType.mult
    ADD = mybir.AluOpType.add
    P = _P

    assert BR % 8 == 0 and H % BR == 0 and W % 8 == 0

    nc = bacc.Bacc("TRN2", target_bir_lowering=False, debug=False)
    x = nc.dram_tensor("x", [P, H * W], dt, kind="ExternalInput")

    dims = {lvl: (H >> lvl, W >> lvl) for lvl in (1, 2, 3)}
    outs = {}
    for lvl in (1, 2, 3):
        h_, w_ = dims[lvl]
        for nm in ("lh", "hl", "hh"):
            outs[f"{nm}{lvl}"] = nc.dram_tensor(
                f"{nm}{lvl}", [P, h_ * w_], dt, kind="ExternalOutput"
            )
    outs["l3"] = nc.dram_tensor("l3", [P, dims[3][0] * dims[3][1]], dt,
                                kind="ExternalOutput")

    nbands = H // BR

    with tile.TileContext(nc) as tc:
        with (
            tc.tile_pool(name="xin", bufs=2) as xp,
            tc.tile_pool(name="uwork", bufs=2) as up,
            tc.tile_pool(name="llp", bufs=1) as llp,
            tc.tile_pool(name="det1", bufs=2) as detp,
            tc.tile_pool(name="acc", bufs=1) as accp,
        ):
            r3f, c3f = dims[3]
            l3_acc = {nm: accp.tile([P, r3f * c3f], dt, tag=f"acc_{nm}",
                                    name=f"acc_{nm}")
                      for nm in ("l3", "lh3", "hl3", "hh3")}
            # level-2 detail half-image accumulators: turns 24 x 256KB DMAs
            # into 6 x 1MB ones
            r2f, c2f = dims[2]
            d2_acc = {nm: accp.tile([P, (r2f // 2) * c2f], dt, tag=f"d2acc_{nm}",
                                    name=f"d2acc_{nm}")
                      for nm in ("lh", "hl", "hh")}

            # Ratio form: a*E + b*O = a*(E + (b/a)*O). Vertical passes and the
            # ll chain keep results UNSCALED (scale folded forward); details
            # get the accumulated scale baked into their scalar-engine
            # prescale. Requires l0, h0 != 0 (true for the Haar bank and any
            # real filter; asserted in kernel()).
            r_l = l1 / l0
            r_h = h1 / h0
            # sigma[lvl] = scale of the unscaled ll' input entering level lvl
            sigma = {1: 1.0, 2: l0 * l0, 3: (l0 * l0) ** 2}

            def do_level(src, R, C, lvl, dst_ll, dst_lh, dst_hl, dst_hh):
                # src: AP [P, R*C] (unscaled by sigma[lvl]); dsts: APs
                # [P, (R//2)*(C//2)]. dst_ll left unscaled; details exact.
                r, c = R // 2, C // 2
                s = sigma[lvl]
                pairs = src.rearrange("p (r two c) -> p r two c", two=2, c=C)
                E = pairs[:, :, 0, :]
                O = pairs[:, :, 1, :]

                # vertical: Vlo' = E + r_l*O ; Vhi' = E + r_h*O
                u = up.tile([P, r * C], dt, tag=f"u{C}", name=f"u{C}a")
                uv = u[:].rearrange("p (r c) -> p r c", c=C)
                nc.vector.scalar_tensor_tensor(uv, O, r_l, E, MULT, ADD)
                u2 = up.tile([P, r * C], dt, tag=f"u{C}", name=f"u{C}b")
                u2v = u2[:].rearrange("p (r c) -> p r c", c=C)
                nc.vector.scalar_tensor_tensor(u2v, O, r_h, E, MULT, ADD)

                def cols(ap):
                    veo = ap.rearrange("p (r c two) -> p r c two", two=2, c=c)
                    return veo[:, :, :, 0], veo[:, :, :, 1]

                Ve, Vo = cols(u[:])
                We, Wo = cols(u2[:])

                # ll' = Ve + r_l*Vo (unscaled by s*l0^2, folded forward)
                ll_v = dst_ll.rearrange("p (r c) -> p r c", c=c)
                nc.vector.scalar_tensor_tensor(ll_v, Vo, r_l, Ve, MULT, ADD)
                if lvl == 3:
                    # final ll-chain scale applied per band (l3 is an output)
                    nc.scalar.mul(ll_v, ll_v, s * l0 * l0)

                # details: d = (s*f0*v0)*Ve + (s*f0*v1)*Vo, exact scales
                for dst, Xe, Xo, f0, f1 in (
                    (dst_hl, Ve, Vo, l0 * h0, l0 * h1),
                    (dst_lh, We, Wo, h0 * l0, h0 * l1),
                    (dst_hh, We, Wo, h0 * h0, h0 * h1),
                ):
                    dv = dst.rearrange("p (r c) -> p r c", c=c)
                    nc.scalar.mul(dv, Xe, s * f0)
                    nc.vector.scalar_tensor_tensor(dv, Xo, s * f1, dv, MULT, ADD)

            # Taper the first and last bands so the pipeline ramps in
            # quickly (vector starts after a 2MB DMA instead of 4MB) and the
            # post-compute DMA tail is halved.
            if nbands >= 2 and BR // 2 >= 8:
                band_rows = [BR // 2, BR // 2] + [BR] * (nbands - 2) + [BR // 2, BR // 2]
            else:
                band_rows = [BR] * nbands
            assert sum(band_rows) == H
            row0 = 0
            l3_flush_done = 0
            l3_rows_total = 0
            for bi, R in enumerate(band_rows):
                X = xp.tile([P, R * W], dt, tag="X", name="X")
                nc.sync.dma_start(X[:], x[:, row0 * W:(row0 + R) * W])

                r1b, c1 = R // 2, W // 2
                r2b, c2 = r1b // 2, c1 // 2
                r3b, c3 = r2b // 2, c2 // 2
                ll1 = llp.tile([P, r1b * c1], dt, tag="ll1", name="ll1")
                d1 = {nm: detp.tile([P, r1b * c1], dt, tag=f"{nm}1", name=f"{nm}1t")
                      for nm in ("lh", "hl", "hh")}
                do_level(X[:], R, W, 1, ll1[:], d1["lh"][:], d1["hl"][:], d1["hh"][:])
                o1 = (row0 // 2) * c1
                for nm in ("lh", "hl", "hh"):
                    nc.sync.dma_start(outs[f"{nm}1"][:, o1:o1 + r1b * c1],
                                      d1[nm][:])

                ll2 = llp.tile([P, r2b * c2], dt, tag="ll2", name="ll2")
                o2 = (row0 // 4) * c2
                half2 = (H // 8) * c2  # elems per half-image at level 2
                a2 = o2 % half2
                d2sl = {nm: d2_acc[nm][:, a2:a2 + r2b * c2]
                        for nm in ("lh", "hl", "hh")}
                do_level(ll1[:], r1b, c1, 2, ll2[:], d2sl["lh"], d2sl["hl"],
                         d2sl["hh"])
                if row0 + R in (H // 2, H):
                    base2 = o2 + r2b * c2 - half2
                    for nm in ("lh", "hl", "hh"):
                        nc.sync.dma_start(outs[f"{nm}2"][:, base2:base2 + half2],
                                          d2_acc[nm][:])

                o3 = (row0 // 8) * c3
                sl = slice(o3, o3 + r3b * c3)
                do_level(ll2[:], r2b, c2, 3, l3_acc["l3"][:, sl], l3_acc["lh3"][:, sl],
                         l3_acc["hl3"][:, sl], l3_acc["hh3"][:, sl])
                l3_rows_total = o3 + r3b * c3

                # flush level-3 accumulators halfway and at the end so the
                # final DMAs are small and overlap earlier compute
                if bi == len(band_rows) // 2 - 1 or bi == len(band_rows) - 1:
                    for nm in ("l3", "lh3", "hl3", "hh3"):
                        nc.sync.dma_start(outs[nm][:, l3_flush_done:l3_rows_total],
                                          l3_acc[nm][:, l3_flush_done:l3_rows_total])
                    l3_flush_done = l3_rows_total
                row0 += R

    nc.compile()
    return nc


_CACHE = {}


def _get_nc(key, l0, l1, h0, h1, H, W):
    if key not in _CACHE:
        _CACHE[key] = build_dwt_nc(l0, l1, h0, h1, H, W, _BR)
    return _CACHE[key]


def kernel(x, dec_lo, dec_hi):
    from concourse.bass_utils import run_bass_kernel_spmd

    x = np.ascontiguousarray(np.asarray(x), dtype=np.float32)
    lo = np.asarray(dec_lo, dtype=np.float32).reshape(-1)
    hi = np.asarray(dec_hi, dtype=np.float32).reshape(-1)
    l0, l1 = float(lo[0]), float(lo[1])
    h0, h1 = float(hi[0]), float(hi[1])
    assert abs(l0) > 1e-20 and abs(h0) > 1e-20, (l0, h0)
    B, C, H, W = x.shape
    assert (B * C) == _P * _NCORES, (B, C)

    nc = _get_nc((l0, l1, h0, h1, H, W), l0, l1, h0, h1, H, W)

    xf = x.reshape(B * C, H * W)
    in_maps = [{"x": xf[c * _P:(c + 1) * _P]} for c in range(_NCORES)]
    res = run_bass_kernel_spmd(nc, in_maps, core_ids=list(range(_NCORES))).results

    def g(name, lvl):
        h_, w_ = H >> lvl, W >> lvl
        return np.concatenate([res[c][name] for c in range(_NCORES)],
                              axis=0).reshape(B, C, h_, w_)

    l3 = g("l3", 3)
    lh3, hl3, hh3 = g("lh3", 3), g("hl3", 3), g("hh3", 3)
    lh2, hl2, hh2 = g("lh2", 2), g("hl2", 2), g("hh2", 2)
    lh1, hl1, hh1 = g("lh1", 1), g("hl1", 1), g("hh1", 1)
    return (l3, (lh3, hl3, hh3), (lh2, hl2, hh2), (lh1, hl1, hh1))


# revision 20
# speedup vs baseline: 1.0416x; 1.0276x over previous
"""Trainium2 Bass kernel: 3-level 2D DWT (depthwise 2x2 stride-2 conv with
separable 2-tap filters, edge padding degenerates to none for even sizes).

Data-parallel over the 1024 (batch*channel) images: 128 images per NeuronCore
across 8 cores. On each core the SBUF partition dim indexes the 128 images and
the free dim holds pixels. Input is streamed in bands of BR=32 rows; since the
2-tap stride-2 transform is non-overlapping, each band flows through all 3
levels entirely on-chip (level-1 details DMA out per band, level-2 details per
band, level-3 results accumulate in SBUF and DMA at the end).

Per level, with E/O = even/odd rows and (for a 2-tap filter [f0, f1]):
    Vlo = l0*E + l1*O          Vhi = h0*E + h1*O      (vertical pass)
    ll  = l0*Vlo_e + l1*Vlo_o  hl = h0*Vlo_e + h1*Vlo_o
    lh  = l0*Vhi_e + l1*Vhi_o  hh = h0*Vhi_e + h1*Vhi_o  (horizontal pass)
Each a*X + b*Y runs as ONE fused scalar_tensor_tensor on VectorE via the
ratio form a*(X + (b/a)*Y) with scales folded forward along the ll chain;
detail outputs get their exact scale via a ScalarE prescale. The kernel is
DMA-bound (memory regime): HBM traffic is the irreducible 33.5MB in +
33.5MB out per core, measured ~335 GB/s sustained.
"""

import numpy as np

_B, _C, _H, _W = 16, 64, 256, 256
_NCORES = 8
_P = 128  # partitions = images per core
_BR = 32  # input rows per band


def build_dwt_nc(l0, l1, h0, h1, H, W, BR):
    """Build the per-core Bass module. Input x: [128, H*W] fp32 (one image per
    partition, row-major). Outputs: lh/hl/hh at levels 1..3 plus l3."""
    import concourse.bacc as bacc
    import concourse.mybir as mybir
    from concourse import tile

    dt = mybir.dt.float32
    MULT = mybir.AluOpType.mult
    ADD = mybir.AluOpType.add
    P = _P

    assert BR % 8 == 0 and H % BR == 0 and W % 8 == 0

    nc = bacc.Bacc("TRN2", target_bir_lowering=False, debug=False)
    x = nc.dram_tensor("x", [P, H * W], dt, kind="ExternalInput")

    dims = {lvl: (H >> lvl, W >> lvl) for lvl in (1, 2, 3)}
    outs = {}
    for lvl in (1, 2, 3):
        h_, w_ = dims[lvl]
        for nm in ("lh", "hl", "hh"):
            outs[f"{nm}{lvl}"] = nc.dram_tensor(
                f"{nm}{lvl}", [P, h_ * w_], dt, kind="ExternalOutput"
            )
    outs["l3"] = nc.dram_tensor("l3", [P, dims[3][0] * dims[3][1]], dt,
                                kind="ExternalOutput")

    nbands = H // BR

    with tile.TileContext(nc) as tc:
        with (
            tc.tile_pool(name="xin", bufs=2) as xp,
            tc.tile_pool(name="uwork", bufs=2) as up,
            tc.tile_pool(name="llp", bufs=1) as llp,
            tc.tile_pool(name="det1", bufs=2) as detp,
            tc.tile_pool(name="det2", bufs=1) as det2p,
            tc.tile_pool(name="acc", bufs=1) as accp,
        ):
            r3f, c3f = dims[3]
            l3_acc = {nm: accp.tile([P, r3f * c3f], dt, tag=f"acc_{nm}",
                                    name=f"acc_{nm}")
                      for nm in ("l3", "lh3", "hl3", "hh3")}

            # Ratio form: a*E + b*O = a*(E + (b/a)*O). Vertical passes and the
            # ll chain keep results UNSCALED (scale folded forward); details
            # get the accumulated scale baked into their scalar-engine
            # prescale. Requires l0, h0 != 0 (true for the Haar bank and any
            # real filter; asserted in kernel()).
            r_l = l1 / l0
            r_h = h1 / h0
            # sigma[lvl] = scale of the unscaled ll' input entering level lvl
            sigma = {1: 1.0, 2: l0 * l0, 3: (l0 * l0) ** 2}

            def do_level(src, R, C, lvl, dst_ll, dst_lh, dst_hl, dst_hh):
                # src: AP [P, R*C] (unscaled by sigma[lvl]); dsts: APs
                # [P, (R//2)*(C//2)]. dst_ll left unscaled; details exact.
                r, c = R // 2, C // 2
                s = sigma[lvl]
                pairs = src.rearrange("p (r two c) -> p r two c", two=2, c=C)
                E = pairs[:, :, 0, :]
                O = pairs[:, :, 1, :]

                # vertical: Vlo' = E + r_l*O ; Vhi' = E + r_h*O
                u = up.tile([P, r * C], dt, tag=f"u{C}", name=f"u{C}a")
                uv = u[:].rearrange("p (r c) -> p r c", c=C)
                nc.vector.scalar_tensor_tensor(uv, O, r_l, E, MULT, ADD)
                u2 = up.tile([P, r * C], dt, tag=f"u{C}", name=f"u{C}b")
                u2v = u2[:].rearrange("p (r c) -> p r c", c=C)
                nc.vector.scalar_tensor_tensor(u2v, O, r_h, E, MULT, ADD)

                def cols(ap):
                    veo = ap.rearrange("p (r c two) -> p r c two", two=2, c=c)
                    return veo[:, :, :, 0], veo[:, :, :, 1]

                Ve, Vo = cols(u[:])
                We, Wo = cols(u2[:])

                # ll' = Ve + r_l*Vo (unscaled by s*l0^2, folded forward)
                ll_v = dst_ll.rearrange("p (r c) -> p r c", c=c)
                nc.vector.scalar_tensor_tensor(ll_v, Vo, r_l, Ve, MULT, ADD)
                if lvl == 3:
                    # final ll-chain scale applied per band (l3 is an output)
                    nc.scalar.mul(ll_v, ll_v, s * l0 * l0)

                # details: d = (s*f0*v0)*Ve + (s*f0*v1)*Vo, exact scales
                for dst, Xe, Xo, f0, f1 in (
                    (dst_hl, Ve, Vo, l0 * h0, l0 * h1),
                    (dst_lh, We, Wo, h0 * l0, h0 * l1),
                    (dst_hh, We, Wo, h0 * h0, h0 * h1),
                ):
                    dv = dst.rearrange("p (r c) -> p r c", c=c)
                    nc.scalar.mul(dv, Xe, s * f0)
                    nc.vector.scalar_tensor_tensor(dv, Xo, s * f1, dv, MULT, ADD)

            # Taper the first and last bands so the pipeline ramps in
            # quickly (vector starts after a 2MB DMA instead of 4MB) and the
            # post-compute DMA tail is halved.
            if nbands >= 2 and BR // 2 >= 8:
                band_rows = [BR // 2, BR // 2] + [BR] * (nbands - 2) + [BR // 2, BR // 2]
            else:
                band_rows = [BR] * nbands
            assert sum(band_rows) == H
            row0 = 0
            l3_flush_done = 0
            l3_rows_total = 0
            for bi, R in enumerate(band_rows):
                X = xp.tile([P, R * W], dt, tag="X", name="X")
                nc.sync.dma_start(X[:], x[:, row0 * W:(row0 + R) * W])

                r1b, c1 = R // 2, W // 2
                r2b, c2 = r1b // 2, c1 // 2
                r3b, c3 = r2b // 2, c2 // 2
                ll1 = llp.tile([P, r1b * c1], dt, tag="ll1", name="ll1")
                d1 = {nm: detp.tile([P, r1b * c1], dt, tag=f"{nm}1", name=f"{nm}1t")
                      for nm in ("lh", "hl", "hh")}
                do_level(X[:], R, W, 1, ll1[:], d1["lh"][:], d1["hl"][:], d1["hh"][:])
                o1 = (row0 // 2) * c1
                for nm in ("lh", "hl", "hh"):
                    nc.sync.dma_start(outs[f"{nm}1"][:, o1:o1 + r1b * c1],
                                      d1[nm][:])

                ll2 = llp.tile([P, r2b * c2], dt, tag="ll2", name="ll2")
                d2 = {nm: det2p.tile([P, r2b * c2], dt, tag=f"{nm}2", name=f"{nm}2t")
                      for nm in ("lh", "hl", "hh")}
                do_level(ll1[:], r1b, c1, 2, ll2[:], d2["lh"][:], d2["hl"][:], d2["hh"][:])
                o2 = (row0 // 4) * c2
                for nm in ("lh", "hl", "hh"):
                    nc.sync.dma_start(outs[f"{nm}2"][:, o2:o2 + r2b * c2],
                                      d2[nm][:])

                o3 = (row0 // 8) * c3
                sl = slice(o3, o3 + r3b * c3)
                do_level(ll2[:], r2b, c2, 3, l3_acc["l3"][:, sl], l3_acc["lh3"][:, sl],
                         l3_acc["hl3"][:, sl], l3_acc["hh3"][:, sl])
                l3_rows_total = o3 + r3b * c3

                # flush level-3 accumulators halfway and at the end so the
                # final DMAs are small and overlap earlier compute
                if bi == len(band_rows) // 2 - 1 or bi == len(band_rows) - 1:
                    for nm in ("l3", "lh3", "hl3", "hh3"):
                        nc.sync.dma_start(outs[nm][:, l3_flush_done:l3_rows_total],
                                          l3_acc[nm][:, l3_flush_done:l3_rows_total])
                    l3_flush_done = l3_rows_total
                row0 += R

    nc.compile()
    return nc


_CACHE = {}


def _get_nc(key, l0, l1, h0, h1, H, W):
    if key not in _CACHE:
        _CACHE[key] = build_dwt_nc(l0, l1, h0, h1, H, W, _BR)
    return _CACHE[key]


def kernel(x, dec_lo, dec_hi):
    from concourse.bass_utils import run_bass_kernel_spmd

    x = np.ascontiguousarray(np.asarray(x), dtype=np.float32)
    lo = np.asarray(dec_lo, dtype=np.float32).reshape(-1)
    hi = np.asarray(dec_hi, dtype=np.float32).reshape(-1)
    l0, l1 = float(lo[0]), float(lo[1])
    h0, h1 = float(hi[0]), float(hi[1])
    assert abs(l0) > 1e-20 and abs(h0) > 1e-20, (l0, h0)
    B, C, H, W = x.shape
    assert (B * C) == _P * _NCORES, (B, C)

    nc = _get_nc((l0, l1, h0, h1, H, W), l0, l1, h0, h1, H, W)

    xf = x.reshape(B * C, H * W)
    in_maps = [{"x": xf[c * _P:(c + 1) * _P]} for c in range(_NCORES)]
    res = run_bass_kernel_spmd(nc, in_maps, core_ids=list(range(_NCORES))).results

    def g(name, lvl):
        h_, w_ = H >> lvl, W >> lvl
        return np.concatenate([res[c][name] for c in range(_NCORES)],
                              axis=0).reshape(B, C, h_, w_)

    l3 = g("l3", 3)
    lh3, hl3, hh3 = g("lh3", 3), g("hl3", 3), g("hh3", 3)
    lh2, hl2, hh2 = g("lh2", 2), g("hl2", 2), g("hh2", 2)
    lh1, hl1, hh1 = g("lh1", 1), g("hl1", 1), g("hh1", 1)
    return (l3, (lh3, hl3, hh3), (lh2, hl2, hh2), (lh1, hl1, hh1))
